# revision 7
# baseline (speedup 1.0000x reference)
"""Trainium2 Bass kernel for CNN-BiLSTM encoder/decoder (nn_CNN_BiLSTM_AttenQ).

Data-parallel over batch: B=128 sharded 8 ways (16 samples/core), weights
replicated, no collectives. Per core:
  encoder conv (matmul over host-im2col patches) ->
  4 sequential LSTM passes (2 layers x 2 dirs, h/c carried across passes) ->
  decoder conv stack (convs/convTs as tap-accumulated matmuls).

Layouts: activations channel-on-partition, batch-major free [C, (b, t)].
LSTM gates computed transposed: PSUM [128, 4, CH, BL] with gate chunk order
[i, f, o, g]; i,f,o pre-scaled by 0.25 (+0.5 via bias preload) so
hard-sigmoid == clamp01.
"""

import os
import numpy as np
import ml_dtypes

import concourse.mybir as mybir
import concourse.tile as tile
from concourse import bacc
from concourse import dve_ops
from concourse.dve_spec import (Spec, Src0, Src1, C0, C1, One, relu, minn,
                                lower, _has_src1)
from concourse.dve_uop import DveOpSpec
from concourse.bass_utils import run_bass_kernel_spmd
from contextlib import ExitStack


def _register_dve_op(name, body, ref):
    """Author a custom DVE op at runtime (sha pinned from our own lowering)."""
    for op in dve_ops.OPS:
        if op.name == name:
            return op
    spec = Spec(body=body, reference=ref)
    op = dve_ops.DveOp(name, spec, subdim=False, uops_sha={})
    dve_ops._SUB_OPCODE_FOR_NAME[name] = max(dve_ops._SUB_OPCODE_FOR_NAME.values()) + 1
    dve_ops.OPS.append(op)
    dve_ops.CUSTOM_DVE_SPECS[name] = spec
    for ver in ("v3", "v4"):
        uops = lower(spec, ver=ver)
        op.uops_sha[ver] = DveOpSpec(
            name=name, opcode=dve_ops.get_dve_sub_opcode(name), uops=uops,
            rd1_en=_has_src1(spec)).sha(ver)
    return op


# out = clamp01(in0) * in1
CLAMP_MUL = _register_dve_op(
    "ANT_CLAMP_MUL", minn(relu(Src0), One) * Src1,
    lambda in0, in1, s0, s1, imm2: np.minimum(np.maximum(in0, 0), 1) * in1)
# out = clamp01(in0*s0 + s1) * clamp01(in1)
HSIG_MUL = _register_dve_op(
    "ANT_HSIG_MUL",
    minn(relu(Src0 * C0 + C1), One) * minn(relu(Src1), One),
    lambda in0, in1, s0, s1, imm2: np.minimum(np.maximum(in0 * s0 + s1, 0), 1)
    * np.minimum(np.maximum(in1, 0), 1))

F32 = mybir.dt.float32
BF16 = mybir.dt.bfloat16
AF = mybir.ActivationFunctionType
OP = mybir.AluOpType
BFNP = ml_dtypes.bfloat16

B, T, C, HS = 128, 2048, 128, 128
N_CORES = 8
BL = B // N_CORES          # 16 samples per core
L = T // 4                 # 512 encoder output length
CH = 32                    # recurrence chunk length (timesteps)
NCH = L // CH
L1 = 2 * L                 # 1024
L2 = T                     # 2048
PAD = 4                    # halo pad per batch segment in decoder buffers
EPS = 1e-5

# convT taps: out[2m+r] += x[m+delta] @ w[:, :, k]  -> list of (delta, k)
CONVT_TAPS = {0: [(1, 1), (0, 3), (-1, 5), (-2, 7)],
              1: [(2, 0), (1, 2), (0, 4), (-1, 6)]}
K5_TAPS = [(k - 2, k) for k in range(5)]


def _bf(x):
    return np.ascontiguousarray(np.asarray(x, np.float32).astype(BFNP))


def _f32(x):
    return np.ascontiguousarray(np.asarray(x, np.float32))


def _prep_dir(W, U, b):
    perm = [0, 1, 3, 2]  # i,f,g,o -> i,f,o,g
    Wp = np.concatenate([W[:, j * HS:(j + 1) * HS] for j in perm], 1).astype(np.float64)
    Up = np.concatenate([U[:, j * HS:(j + 1) * HS] for j in perm], 1).astype(np.float64)
    bp = np.concatenate([b[j * HS:(j + 1) * HS] for j in perm]).astype(np.float64)
    Wp[:, :3 * HS] *= 0.25
    Up[:, :3 * HS] *= 0.25
    bp[:3 * HS] = 0.25 * bp[:3 * HS] + 0.5
    bA = np.stack([bp[0:HS], bp[HS:2 * HS]])
    bB = np.stack([bp[2 * HS:3 * HS], bp[3 * HS:4 * HS]])
    return _bf(Wp), _bf(Up), _bf(bA), _bf(bB)


def _prep_weights(inp):
    w = {}
    w["w1"] = _bf(np.asarray(inp["conv1_w"])[:, 0, :].T)            # [8,128]
    w["s1"] = _f32(np.asarray(inp["bn1_g"]) / np.sqrt(1.0 + EPS))[:, None]
    w["b1"] = _f32(np.asarray(inp["bn1_b"]))[:, None]
    for tag, (Wk, Uk, bk) in {
        "0f": ("W0f", "U0f", "b0f"), "0r": ("W0r", "U0r", "b0r"),
        "1f": ("W1f", "U1f", "b1f"), "1r": ("W1r", "U1r", "b1r"),
    }.items():
        Wp, Up, bA, bB = _prep_dir(np.asarray(inp[Wk], np.float32),
                                   np.asarray(inp[Uk], np.float32),
                                   np.asarray(inp[bk], np.float32))
        if Wp.shape[0] == 2 * HS:
            w[f"W{tag}a"] = np.ascontiguousarray(Wp[:HS])
            w[f"W{tag}b"] = np.ascontiguousarray(Wp[HS:])
        else:
            w[f"W{tag}a"] = Wp
        w[f"U{tag}"], w[f"bA{tag}"], w[f"bB{tag}"] = Up, bA, bB
    u1w = np.asarray(inp["up1_w"], np.float32)                      # [256,128,8]
    u1 = np.zeros((HS, 16 * HS), np.float32)
    for r in (0, 1):
        for ti, (_, k) in enumerate(CONVT_TAPS[r]):
            for ch in (0, 1):
                blk = (r * 4 + ti) * 2 + ch
                u1[:, blk * HS:(blk + 1) * HS] = u1w[ch * HS:(ch + 1) * HS, :, k]
    w["u1"] = _bf(u1)
    d11w = np.asarray(inp["d11_w"], np.float32)
    w["d11"] = _bf(np.concatenate([d11w[:, :, k].T for k in range(5)], 1))
    w["s11"] = _f32(np.asarray(inp["bn11_g"]) / np.sqrt(1.0 + EPS))[:, None]
    w["b11"] = _f32(np.asarray(inp["bn11_b"]))[:, None]
    d12w = np.asarray(inp["d12_w"], np.float32)
    w["d12"] = _bf(np.concatenate([d12w[:, :, k].T for k in range(5)], 1))
    w["s12"] = _f32(np.asarray(inp["bn12_g"]) / np.sqrt(1.0 + EPS))[:, None]
    w["b12"] = _f32(np.asarray(inp["bn12_b"]))[:, None]
    u2w = np.asarray(inp["up2_w"], np.float32)                      # [64,32,8]
    u2 = np.zeros((64, 8 * 32), np.float32)
    for r in (0, 1):
        for ti, (_, k) in enumerate(CONVT_TAPS[r]):
            blk = r * 4 + ti
            u2[:, blk * 32:(blk + 1) * 32] = u2w[:, :, k]
    w["u2"] = _bf(u2)
    d21w = np.asarray(inp["d21_w"], np.float32)
    w["d21"] = _bf(np.concatenate([d21w[:, :, k].T for k in range(5)], 1))
    w["s21"] = _f32(np.asarray(inp["bn21_g"]) / np.sqrt(1.0 + EPS))[:, None]
    w["b21"] = _f32(np.asarray(inp["bn21_b"]))[:, None]
    d22w = np.asarray(inp["d22_w"], np.float32)
    w["d22"] = _bf(np.concatenate([d22w[:, :, k].T for k in range(5)], 1))
    w["s22"] = _f32(np.asarray(inp["bn22_g"]) / np.sqrt(1.0 + EPS))[:, None]
    w["b22"] = _f32(np.asarray(inp["bn22_b"]))[:, None]
    blk = CH * (BL // NS)          # 256 cols per gate block
    ind = np.zeros((2, 2 * blk), np.float32)
    ind[0, :blk] = 1.0
    ind[1, blk:] = 1.0
    w["ind"] = _bf(ind)
    return w


def _prep_xcol(x_shard):
    xp = np.pad(np.asarray(x_shard, np.float32)[:, 0, :], ((0, 0), (3, 4)))
    cols = np.stack([xp[:, k:k + 4 * L:4] for k in range(8)])       # [8,BL,L]
    return _bf(cols.reshape(8, BL * L))


_WSPEC = [
    ("w1", [8, HS], BF16), ("s1", [HS, 1], F32), ("b1", [HS, 1], F32),
    ("W0fa", [HS, 512], BF16), ("U0f", [HS, 512], BF16),
    ("bA0f", [2, HS], BF16), ("bB0f", [2, HS], BF16),
    ("W0ra", [HS, 512], BF16), ("U0r", [HS, 512], BF16),
    ("bA0r", [2, HS], BF16), ("bB0r", [2, HS], BF16),
    ("W1fa", [HS, 512], BF16), ("W1fb", [HS, 512], BF16), ("U1f", [HS, 512], BF16),
    ("bA1f", [2, HS], BF16), ("bB1f", [2, HS], BF16),
    ("W1ra", [HS, 512], BF16), ("W1rb", [HS, 512], BF16), ("U1r", [HS, 512], BF16),
    ("bA1r", [2, HS], BF16), ("bB1r", [2, HS], BF16),
    ("u1", [HS, 16 * HS], BF16),
    ("d11", [HS, 640], BF16), ("s11", [HS, 1], F32), ("b11", [HS, 1], F32),
    ("d12", [HS, 320], BF16), ("s12", [64, 1], F32), ("b12", [64, 1], F32),
    ("u2", [64, 256], BF16),
    ("d21", [32, 160], BF16), ("s21", [32, 1], F32), ("b21", [32, 1], F32),
    ("d22", [32, 20], BF16), ("s22", [4, 1], F32), ("b22", [4, 1], F32),
    ("ind", [2, 512], BF16),
]


NS = 2                      # phase-offset streams per core
SB = BL // NS               # 8 samples per stream


def _emit_pass(nc, pools, wt, ins, Hout, h_aps, state, reverse, t_off):
    """One LSTM direction (L steps, NCH chunks), NS interleaved batch
    streams. ins: input APs [128,BL,L]. Hout: [128,BL,Lseg]; h written at
    [:,bs,t_off+t]. state: persistent [HS,3,NS,SB] fp32 SBUF tile holding
    (o_hat, g_hat, c) rows per stream. Returns updated h_aps.

    Per step per stream the work is spread over 4 engines:
      PE:     4 U-gate MMs (streams paired per gate to share LDWEIGHTS)
      scalar: (o_hat, g_hat) = relu(pg[o,g])  PSUM->SBUF, one op
      DVE:    (t1, cf) = clamp01(pg[i,f]) * (g_hat, c)  one paired op
      gpsimd: c = t1 + cf  (SBUF only)
      DVE:    h = hsig(c) * clamp01(o_hat)  all-SBUF op
    """
    psum_pool, tmp = pools["psum_r"], pools["tmp"]
    ind = pools["ind"]
    n_in = 2 if "Wb" in wt else 1
    h_aps = list(h_aps)

    def gemm_pieces(ci, si):
        """Allocate pg and return (pg, T0, [thunk, ...]) — small PE emission
        pieces to spread between steps (PE is in-order; keep pieces short)."""
        T0 = ci * CH if not reverse else L - (ci + 1) * CH
        pg = psum_pool.tile([HS, 4, CH, SB], F32, tag=f"pg{si}",
                            name=f"pg{si}")

        def bias():
            nc.tensor.matmul(pg[:, 0:2, :, :], wt["bA"][:], ind[:],
                             start=True, stop=False)
            nc.tensor.matmul(pg[:, 2:4, :, :], wt["bB"][:], ind[:],
                             start=True, stop=False)

        pieces = [bias]
        for j in range(4):
            def wmm(j=j):
                for idx in range(n_in):
                    Wm = wt["Wa"] if idx == 0 else wt["Wb"]
                    rhs = ins[idx].rearrange("p b t -> p t b")[
                        :, T0:T0 + CH, si * SB:(si + 1) * SB]
                    nc.tensor.matmul(pg[:, j, :, :],
                                     Wm[:, j * HS:(j + 1) * HS],
                                     rhs, start=False, stop=False)
            pieces.append(wmm)
        return pg, T0, pieces

    def run_all(pieces_list):
        for _, _, pieces in pieces_list:
            for p in pieces:
                p()

    # gate order in pg: [i, f, o, g]; emit (o, g) first so the scalar relu
    # can start while (i, f) stream; streams paired per gate for LDW reuse.
    GATE_SEQ = (2, 3, 0, 1)

    nxt = [gemm_pieces(0, si) for si in range(NS)]
    run_all(nxt)
    for ci in range(NCH):
        pgs = [pg for pg, _, _ in nxt]
        T0 = nxt[0][1]
        if ci + 1 < NCH:
            nxt = [gemm_pieces(ci + 1, si) for si in range(NS)]
            todo = [p for _, _, pieces in nxt for p in pieces]
        else:
            nxt, todo = [], []
        for s in range(CH):
            tl = s if not reverse else CH - 1 - s
            t = T0 + tl
            T = tmp.tile([HS, 2, NS, SB], F32, tag="T", name="T")
            for j in GATE_SEQ:
                for si in range(NS):
                    nc.tensor.matmul(pgs[si][:, j, tl, :],
                                     wt["U"][:, j * HS:(j + 1) * HS],
                                     h_aps[si], start=False, stop=(j == 1))
            for si in range(NS):
                # (o_hat, g_hat) <- relu(o', g); clamp01(relu(o'))==clamp01(o')
                nc.scalar.activation(state[:, 0:2, si, :],
                                     pgs[si][:, 2:4, tl, :], AF.Relu)
            for si in range(NS):
                # (t1, cf) = clamp01((i', f')) * (g_hat, c)
                nc.vector._custom_dve(CLAMP_MUL, out=T[:, :, si, :],
                                      in0=pgs[si][:, 0:2, tl, :],
                                      in1=state[:, 1:3, si, :])
                nc.gpsimd.tensor_tensor(state[:, 2, si, :], T[:, 0, si, :],
                                        T[:, 1, si, :], OP.add)
                h_aps[si] = Hout[:, si * SB:(si + 1) * SB, t_off + t]
                nc.vector._custom_dve(HSIG_MUL, out=h_aps[si],
                                      in0=state[:, 2, si, :],
                                      in1=state[:, 0, si, :],
                                      s0=0.25, s1=0.5)
            # hide next chunk's gemm pieces behind this step's PE work,
            # one piece every 3rd step so each hides under a DVE chain
            if s % 3 == 2 and s // 3 < len(todo):
                todo[s // 3]()
    return h_aps


def _conv_b(nc, pools, dst, src, lhsT, taps, n_len, scale, bias, out_parts,
            y_b=None, ptag="pg0"):
    """Per-sample K-tap conv: dst [parts, PAD+n_len+PAD] (halo-padded 2D tile)
    = relu(scale*psum+bias). If y_b is given (d22), DMA each tile to DRAM."""
    psum_pool = pools["psum_d"]
    for n0 in range(0, n_len, 512):
        pd = psum_pool.tile([out_parts, 512], F32, tag=ptag, name="pd")
        for i, (delta, blk) in enumerate(taps):
            rhs = src[:, PAD + n0 + delta: PAD + n0 + delta + 512]
            nc.tensor.matmul(pd[:], lhsT[:, blk * out_parts:(blk + 1) * out_parts],
                             rhs, start=(i == 0), stop=(i == len(taps) - 1))
        if y_b is not None:
            ot = pools["otile"].tile([out_parts, 512], F32, tag="otile",
                                     name="ot")
            nc.scalar.activation(ot[:], pd[:], AF.Relu, bias=bias[:],
                                 scale=scale[:])
            nc.sync.dma_start(y_b[:, n0:n0 + 512], ot[:])
        else:
            nc.scalar.activation(dst[:, PAD + n0:PAD + n0 + 512], pd[:],
                                 AF.Relu, bias=bias[:], scale=scale[:])


def _convT_b(nc, pools, dst, srcs, lhsT, out_parts, m_len, ptag="pg0"):
    """Per-sample convT stride 2: dst[:, PAD+2m+r] accumulated over taps and
    input chunks. srcs: list of [parts, PAD+m_len+PAD] APs."""
    psum_pool = pools["psum_d"]
    nchunk = len(srcs)
    dv = dst.rearrange("p (m r) -> p m r", r=2)
    for r in (0, 1):
        for m0 in range(0, m_len, 512):
            pd = psum_pool.tile([out_parts, 512], F32, tag=ptag, name="pd")
            first = True
            for ti, (delta, _) in enumerate(CONVT_TAPS[r]):
                for ch in range(nchunk):
                    blk = (r * 4 + ti) * nchunk + ch
                    rhs = srcs[ch][:, PAD + m0 + delta: PAD + m0 + delta + 512]
                    nc.tensor.matmul(
                        pd[:], lhsT[:, blk * out_parts:(blk + 1) * out_parts],
                        rhs, start=first, stop=(ti == 3 and ch == nchunk - 1))
                    first = False
            # dst col = PAD + 2(m0+m) + r = 2*(PAD//2 + m0 + m) + r
            nc.vector.tensor_copy(dv[:, PAD // 2 + m0: PAD // 2 + m0 + 512, r],
                                  pd[:])


def build_nc():
    nc = bacc.Bacc()
    xcol_d = nc.declare_dram_parameter("xcol", [8, BL * L], BF16, isOutput=False)
    wd = {name: nc.declare_dram_parameter(name, shape, dt, isOutput=False)
          for name, shape, dt in _WSPEC}
    y_d = nc.declare_dram_parameter("y", [BL, 4, L2], F32, isOutput=True)

    with tile.TileContext(nc) as tc:
        with ExitStack() as top:
            wpool = top.enter_context(tc.tile_pool(name="w", bufs=1))
            state = top.enter_context(tc.tile_pool(name="state", bufs=1))
            tmp = top.enter_context(tc.tile_pool(name="tmp", bufs=4))
            psum_r = top.enter_context(tc.tile_pool(name="psum_r", bufs=2,
                                                    space="PSUM"))
            psum_d = psum_r
            otile = top.enter_context(tc.tile_pool(name="otile", bufs=3))

            wt = {}
            for name, shape, dt in _WSPEC:
                wt[name] = wpool.tile(shape, dt, tag=f"w_{name}", name=f"w_{name}")
                nc.sync.dma_start(wt[name][:], wd[name][:])
            xcol = wpool.tile([8, BL * L], BF16, tag="xcol")
            nc.sync.dma_start(xcol[:], xcol_d[:])

            ind = wt["ind"]

            st = state.tile([HS, 3, NS, SB], F32, tag="lstm_state",
                            name="lstm_state")
            nc.gpsimd.memset(st[:, 2, :, :], 0.0)
            hz = state.tile([HS, BL], BF16, tag="hz")
            nc.gpsimd.memset(hz[:], 0.0)

            pools = {"psum_r": psum_r, "psum_d": psum_d, "tmp": tmp,
                     "ind": ind, "otile": otile, "y": y_d}

            bigpool = top.enter_context(tc.tile_pool(name="big", bufs=1))
            dpool = top.enter_context(tc.tile_pool(name="dec", bufs=3))

            E = bigpool.tile([HS, BL, L], BF16, tag="E")
            for b in range(BL):
                pe = psum_d.tile([HS, 512], F32, tag="pg0")
                nc.tensor.matmul(pe[:], wt["w1"][:], xcol[:, b * L:(b + 1) * L],
                                 start=True, stop=True)
                nc.scalar.activation(E[:, b, :], pe[:], AF.Relu,
                                     bias=wt["b1"][:], scale=wt["s1"][:])

            H0F = bigpool.tile([HS, BL, L], BF16, tag="H0F")
            H0R = bigpool.tile([HS, BL, L], BF16, tag="H0R")
            h_aps = [hz[:, si * SB:(si + 1) * SB] for si in range(NS)]
            h_aps = _emit_pass(nc, pools,
                               {"Wa": wt["W0fa"], "U": wt["U0f"],
                                "bA": wt["bA0f"], "bB": wt["bB0f"]},
                               [E[:]], H0F, h_aps, st, False, 0)
            h_aps = _emit_pass(nc, pools,
                               {"Wa": wt["W0ra"], "U": wt["U0r"],
                                "bA": wt["bA0r"], "bB": wt["bB0r"]},
                               [E[:]], H0R, h_aps, st, True, 0)

            LS = L + 2 * PAD
            H1F = bigpool.tile([HS, BL, LS], BF16, tag="H1F")
            H1R = bigpool.tile([HS, BL, LS], BF16, tag="H1R")
            for Hb in (H1F, H1R):
                nc.gpsimd.memset(Hb[:, :, 0:PAD], 0.0)
                nc.gpsimd.memset(Hb[:, :, PAD + L:LS], 0.0)
            h_aps = _emit_pass(nc, pools,
                               {"Wa": wt["W1fa"], "Wb": wt["W1fb"],
                                "U": wt["U1f"], "bA": wt["bA1f"],
                                "bB": wt["bB1f"]},
                               [H0F[:], H0R[:]], H1F, h_aps, st, False, PAD)
            h_aps = _emit_pass(nc, pools,
                               {"Wa": wt["W1ra"], "Wb": wt["W1rb"],
                                "U": wt["U1r"], "bA": wt["bA1r"],
                                "bB": wt["bB1r"]},
                               [H0F[:], H0R[:]], H1R, h_aps, st, True, PAD)

            # ---- decoder: per-sample chain with small ping-pong tiles ----
            S1 = L1 + 2 * PAD
            S2 = L2 + 2 * PAD
            for b in range(BL):
                D1 = dpool.tile([HS, S1], BF16, tag="D1", name="D1")
                nc.gpsimd.memset(D1[:, 0:PAD], 0.0)
                nc.gpsimd.memset(D1[:, PAD + L1:S1], 0.0)
                _convT_b(nc, pools, D1,
                         [H1F[:, b, :], H1R[:, b, :]], wt["u1"], HS, L,
                         ptag="pg0")
                D2 = dpool.tile([HS, S1], BF16, tag="D2", name="D2")
                nc.gpsimd.memset(D2[:, 0:PAD], 0.0)
                nc.gpsimd.memset(D2[:, PAD + L1:S1], 0.0)
                _conv_b(nc, pools, D2, D1, wt["d11"], K5_TAPS, L1,
                        wt["s11"], wt["b11"], HS, ptag="pg1")
                D3 = dpool.tile([64, S1], BF16, tag="D3", name="D3")
                nc.gpsimd.memset(D3[:, 0:PAD], 0.0)
                nc.gpsimd.memset(D3[:, PAD + L1:S1], 0.0)
                _conv_b(nc, pools, D3, D2, wt["d12"], K5_TAPS, L1,
                        wt["s12"], wt["b12"], 64, ptag="pg0")
                D4 = dpool.tile([32, S2], BF16, tag="D4", name="D4")
                nc.gpsimd.memset(D4[:, 0:PAD], 0.0)
                nc.gpsimd.memset(D4[:, PAD + L2:S2], 0.0)
                _convT_b(nc, pools, D4, [D3], wt["u2"], 32, L1, ptag="pg1")
                D5 = dpool.tile([32, S2], BF16, tag="D5", name="D5")
                nc.gpsimd.memset(D5[:, 0:PAD], 0.0)
                nc.gpsimd.memset(D5[:, PAD + L2:S2], 0.0)
                _conv_b(nc, pools, D5, D4, wt["d21"], K5_TAPS, L2,
                        wt["s21"], wt["b21"], 32, ptag="pg0")
                _conv_b(nc, pools, None, D5, wt["d22"], K5_TAPS, L2,
                        wt["s22"], wt["b22"], 4, y_b=y_d[b], ptag="pg1")
    nc.finalize()
    return nc


_NC = None


def _get_nc():
    global _NC
    if _NC is None:
        _NC = build_nc()
    return _NC


def kernel(**inputs):
    nc = _get_nc()
    w = _prep_weights(inputs)
    x = np.asarray(inputs["x"], np.float32)
    in_maps = []
    for c in range(N_CORES):
        m = dict(w)
        m["xcol"] = _prep_xcol(x[c * BL:(c + 1) * BL])
        in_maps.append(m)
    trace = bool(int(os.environ.get("BASS_KERNEL_TRACE", "0")))
    res = run_bass_kernel_spmd(nc, in_maps, list(range(N_CORES)), trace=trace)
    if trace:
        kernel.last_exec_time_ns = res.exec_time_ns
    out = np.concatenate([res.results[i]["y"] for i in range(N_CORES)], axis=0)
    return np.ascontiguousarray(out.astype(np.float32))



# revision 11
# speedup vs baseline: 1.2171x; 1.2171x over previous
"""Trainium2 Bass kernel for CNN-BiLSTM encoder/decoder (nn_CNN_BiLSTM_AttenQ).

Data-parallel over batch: B=128 sharded 8 ways (16 samples/core), weights
replicated, no collectives. Per core:
  encoder conv (matmul over host-im2col patches) ->
  4 sequential LSTM passes (2 layers x 2 dirs, h/c carried across passes) ->
  decoder conv stack (convs/convTs as tap-accumulated matmuls).

Layouts: activations channel-on-partition, batch-major free [C, (b, t)].
LSTM gates computed transposed: PSUM [128, 4, CH, BL] with gate chunk order
[i, f, o, g]; i,f,o pre-scaled by 0.25 (+0.5 via bias preload) so
hard-sigmoid == clamp01.
"""

import os
import numpy as np
import ml_dtypes

import concourse.mybir as mybir
import concourse.tile as tile
from concourse import bacc
from concourse import dve_ops
from concourse.dve_spec import (Spec, Src0, Src1, C0, C1, One, relu, minn,
                                lower, _has_src1)
from concourse.dve_uop import DveOpSpec
from concourse.bass_utils import run_bass_kernel_spmd
from contextlib import ExitStack


def _register_dve_op(name, body, ref):
    """Author a custom DVE op at runtime (sha pinned from our own lowering)."""
    for op in dve_ops.OPS:
        if op.name == name:
            return op
    spec = Spec(body=body, reference=ref)
    op = dve_ops.DveOp(name, spec, subdim=False, uops_sha={})
    dve_ops._SUB_OPCODE_FOR_NAME[name] = max(dve_ops._SUB_OPCODE_FOR_NAME.values()) + 1
    dve_ops.OPS.append(op)
    dve_ops.CUSTOM_DVE_SPECS[name] = spec
    for ver in ("v3", "v4"):
        uops = lower(spec, ver=ver)
        op.uops_sha[ver] = DveOpSpec(
            name=name, opcode=dve_ops.get_dve_sub_opcode(name), uops=uops,
            rd1_en=_has_src1(spec)).sha(ver)
    return op


# out = clamp01(in0) * in1
CLAMP_MUL = _register_dve_op(
    "ANT_CLAMP_MUL", minn(relu(Src0), One) * Src1,
    lambda in0, in1, s0, s1, imm2: np.minimum(np.maximum(in0, 0), 1) * in1)
# out = clamp01(in0*s0 + s1) * clamp01(in1)
HSIG_MUL = _register_dve_op(
    "ANT_HSIG_MUL",
    minn(relu(Src0 * C0 + C1), One) * minn(relu(Src1), One),
    lambda in0, in1, s0, s1, imm2: np.minimum(np.maximum(in0 * s0 + s1, 0), 1)
    * np.minimum(np.maximum(in1, 0), 1))

F32 = mybir.dt.float32
BF16 = mybir.dt.bfloat16
AF = mybir.ActivationFunctionType
OP = mybir.AluOpType
BFNP = ml_dtypes.bfloat16

B, T, C, HS = 128, 2048, 128, 128
N_CORES = 8
BL = B // N_CORES          # 16 samples per core
L = T // 4                 # 512 encoder output length
CH = 32                    # recurrence chunk length (timesteps)
NCH = L // CH
L1 = 2 * L                 # 1024
L2 = T                     # 2048
PAD = 4                    # halo pad per batch segment in decoder buffers
EPS = 1e-5

# convT taps: out[2m+r] += x[m+delta] @ w[:, :, k]  -> list of (delta, k)
CONVT_TAPS = {0: [(1, 1), (0, 3), (-1, 5), (-2, 7)],
              1: [(2, 0), (1, 2), (0, 4), (-1, 6)]}
K5_TAPS = [(k - 2, k) for k in range(5)]


def _bf(x):
    return np.ascontiguousarray(np.asarray(x, np.float32).astype(BFNP))


def _f32(x):
    return np.ascontiguousarray(np.asarray(x, np.float32))


def _prep_dir(W, U, b):
    perm = [0, 1, 3, 2]  # i,f,g,o -> i,f,o,g
    Wp = np.concatenate([W[:, j * HS:(j + 1) * HS] for j in perm], 1).astype(np.float64)
    Up = np.concatenate([U[:, j * HS:(j + 1) * HS] for j in perm], 1).astype(np.float64)
    bp = np.concatenate([b[j * HS:(j + 1) * HS] for j in perm]).astype(np.float64)
    Wp[:, :3 * HS] *= 0.25
    Up[:, :3 * HS] *= 0.25
    bp[:3 * HS] = 0.25 * bp[:3 * HS] + 0.5
    bA = np.stack([bp[0:HS], bp[HS:2 * HS]])
    bB = np.stack([bp[2 * HS:3 * HS], bp[3 * HS:4 * HS]])
    return _bf(Wp), _bf(Up), _bf(bA), _bf(bB)


def _prep_weights(inp):
    w = {}
    w["w1"] = _bf(np.asarray(inp["conv1_w"])[:, 0, :].T)            # [8,128]
    w["s1"] = _f32(np.asarray(inp["bn1_g"]) / np.sqrt(1.0 + EPS))[:, None]
    w["b1"] = _f32(np.asarray(inp["bn1_b"]))[:, None]
    for tag, (Wk, Uk, bk) in {
        "0f": ("W0f", "U0f", "b0f"), "0r": ("W0r", "U0r", "b0r"),
        "1f": ("W1f", "U1f", "b1f"), "1r": ("W1r", "U1r", "b1r"),
    }.items():
        Wp, Up, bA, bB = _prep_dir(np.asarray(inp[Wk], np.float32),
                                   np.asarray(inp[Uk], np.float32),
                                   np.asarray(inp[bk], np.float32))
        if Wp.shape[0] == 2 * HS:
            w[f"W{tag}a"] = np.ascontiguousarray(Wp[:HS])
            w[f"W{tag}b"] = np.ascontiguousarray(Wp[HS:])
        else:
            w[f"W{tag}a"] = Wp
        w[f"U{tag}"], w[f"bA{tag}"], w[f"bB{tag}"] = Up, bA, bB
    u1w = np.asarray(inp["up1_w"], np.float32)                      # [256,128,8]
    u1 = np.zeros((HS, 16 * HS), np.float32)
    for r in (0, 1):
        for ti, (_, k) in enumerate(CONVT_TAPS[r]):
            for ch in (0, 1):
                blk = (r * 4 + ti) * 2 + ch
                u1[:, blk * HS:(blk + 1) * HS] = u1w[ch * HS:(ch + 1) * HS, :, k]
    w["u1"] = _bf(u1)
    d11w = np.asarray(inp["d11_w"], np.float32)
    w["d11"] = _bf(np.concatenate([d11w[:, :, k].T for k in range(5)], 1))
    w["s11"] = _f32(np.asarray(inp["bn11_g"]) / np.sqrt(1.0 + EPS))[:, None]
    w["b11"] = _f32(np.asarray(inp["bn11_b"]))[:, None]
    d12w = np.asarray(inp["d12_w"], np.float32)
    w["d12"] = _bf(np.concatenate([d12w[:, :, k].T for k in range(5)], 1))
    w["s12"] = _f32(np.asarray(inp["bn12_g"]) / np.sqrt(1.0 + EPS))[:, None]
    w["b12"] = _f32(np.asarray(inp["bn12_b"]))[:, None]
    u2w = np.asarray(inp["up2_w"], np.float32)                      # [64,32,8]
    u2 = np.zeros((64, 8 * 32), np.float32)
    for r in (0, 1):
        for ti, (_, k) in enumerate(CONVT_TAPS[r]):
            blk = r * 4 + ti
            u2[:, blk * 32:(blk + 1) * 32] = u2w[:, :, k]
    w["u2"] = _bf(u2)
    d21w = np.asarray(inp["d21_w"], np.float32)
    w["d21"] = _bf(np.concatenate([d21w[:, :, k].T for k in range(5)], 1))
    w["s21"] = _f32(np.asarray(inp["bn21_g"]) / np.sqrt(1.0 + EPS))[:, None]
    w["b21"] = _f32(np.asarray(inp["bn21_b"]))[:, None]
    d22w = np.asarray(inp["d22_w"], np.float32)
    w["d22"] = _bf(np.concatenate([d22w[:, :, k].T for k in range(5)], 1))
    w["s22"] = _f32(np.asarray(inp["bn22_g"]) / np.sqrt(1.0 + EPS))[:, None]
    w["b22"] = _f32(np.asarray(inp["bn22_b"]))[:, None]
    blk = CH * (BL // NS)          # 256 cols per gate block
    ind = np.zeros((2, 2 * blk), np.float32)
    ind[0, :blk] = 1.0
    ind[1, blk:] = 1.0
    w["ind"] = _bf(ind)
    return w


def _prep_xcol(x_shard):
    xp = np.pad(np.asarray(x_shard, np.float32)[:, 0, :], ((0, 0), (3, 4)))
    cols = np.stack([xp[:, k:k + 4 * L:4] for k in range(8)])       # [8,BL,L]
    return _bf(cols.reshape(8, BL * L))


_WSPEC = [
    ("w1", [8, HS], BF16), ("s1", [HS, 1], F32), ("b1", [HS, 1], F32),
    ("W0fa", [HS, 512], BF16), ("U0f", [HS, 512], BF16),
    ("bA0f", [2, HS], BF16), ("bB0f", [2, HS], BF16),
    ("W0ra", [HS, 512], BF16), ("U0r", [HS, 512], BF16),
    ("bA0r", [2, HS], BF16), ("bB0r", [2, HS], BF16),
    ("W1fa", [HS, 512], BF16), ("W1fb", [HS, 512], BF16), ("U1f", [HS, 512], BF16),
    ("bA1f", [2, HS], BF16), ("bB1f", [2, HS], BF16),
    ("W1ra", [HS, 512], BF16), ("W1rb", [HS, 512], BF16), ("U1r", [HS, 512], BF16),
    ("bA1r", [2, HS], BF16), ("bB1r", [2, HS], BF16),
    ("u1", [HS, 16 * HS], BF16),
    ("d11", [HS, 640], BF16), ("s11", [HS, 1], F32), ("b11", [HS, 1], F32),
    ("d12", [HS, 320], BF16), ("s12", [64, 1], F32), ("b12", [64, 1], F32),
    ("u2", [64, 256], BF16),
    ("d21", [32, 160], BF16), ("s21", [32, 1], F32), ("b21", [32, 1], F32),
    ("d22", [32, 20], BF16), ("s22", [4, 1], F32), ("b22", [4, 1], F32),
    ("ind", [2, 512], BF16),
]


NS = 2                      # phase-offset streams per core
SB = BL // NS               # 8 samples per stream


def _emit_pass(nc, pools, wt, ins, Hout, h_aps, state, reverse, t_off):
    """One LSTM direction (L steps, NCH chunks), NS interleaved batch
    streams. ins: input APs [128,L,BL] (t-major). Hout: [128,Lseg,BL]; h
    written at [:,t_off+t,bs]. state: persistent [HS,3,NS,SB] fp32 SBUF
    tile holding (o_hat, g_hat, c) rows per stream. Returns updated h_aps.

    Per step per stream the work is spread over engines:
      PE:     4 U-gate MMs (streams paired per gate to share LDWEIGHTS)
      scalar: (o_hat, g_hat) = relu(pg[o,g])  PSUM->SBUF, one op
      DVE:    (t1, cf) = clamp01(pg[i,f]) * (g_hat, c)  one paired op
      gpsimd: c = t1 + cf  (SBUF only)
      DVE:    h = hsig(c) * clamp01(o_hat)  all-SBUF op
    DVE queue order is [V1_s0, V1_s1, V2_s0, V2_s1] so stream 1's V1 is
    not blocked behind stream 0's pool-dependent V2 (lets streams slide).
    """
    psum_pool, tmp = pools["psum_r"], pools["tmp"]
    ind = pools["ind"]
    n_in = 2 if "Wb" in wt else 1
    h_aps = list(h_aps)

    def gemm_pieces(ci, si):
        """Allocate pg and return (pg, T0, [thunk, ...]) — small PE emission
        pieces to spread between steps (PE is in-order; keep pieces short)."""
        T0 = ci * CH if not reverse else L - (ci + 1) * CH
        pg = psum_pool.tile([HS, 4, CH, SB], F32, tag=f"pg{si}",
                            name=f"pg{si}")

        def bias():
            nc.tensor.matmul(pg[:, 0:2, :, :], wt["bA"][:], ind[:],
                             start=True, stop=False)
            nc.tensor.matmul(pg[:, 2:4, :, :], wt["bB"][:], ind[:],
                             start=True, stop=False)

        pieces = [bias]
        for j in range(4):
            for idx in range(n_in):
                def wmm(j=j, idx=idx):
                    Wm = wt["Wa"] if idx == 0 else wt["Wb"]
                    rhs = ins[idx][:, T0:T0 + CH, si * SB:(si + 1) * SB]
                    nc.tensor.matmul(pg[:, j, :, :],
                                     Wm[:, j * HS:(j + 1) * HS],
                                     rhs, start=False, stop=False)
                pieces.append(wmm)
        return pg, T0, pieces

    def run_all(pieces_list):
        for _, _, pieces in pieces_list:
            for p in pieces:
                p()

    # gate order in pg: [i, f, o, g]; emit (o, g) first so the scalar relu
    # can start while (i, f) stream; streams paired per gate for LDW reuse.
    GATE_SEQ = (2, 3, 0, 1)

    nxt = [gemm_pieces(0, si) for si in range(NS)]
    run_all(nxt)
    for ci in range(NCH):
        pgs = [pg for pg, _, _ in nxt]
        T0 = nxt[0][1]
        if ci + 1 < NCH:
            nxt = [gemm_pieces(ci + 1, si) for si in range(NS)]
            todo = [p for _, _, pieces in nxt for p in pieces]
        else:
            nxt, todo = [], []
        for s in range(CH):
            tl = s if not reverse else CH - 1 - s
            t = T0 + tl
            T = tmp.tile([HS, 2, NS, SB], F32, tag="T", name="T")
            for j in GATE_SEQ:
                for si in range(NS):
                    nc.tensor.matmul(pgs[si][:, j, tl, :],
                                     wt["U"][:, j * HS:(j + 1) * HS],
                                     h_aps[si], start=False, stop=(j == 1))
            # one W@x piece of the next chunk per step, early in the slot
            if s < len(todo):
                todo[s]()
            for si in range(NS):
                # (o_hat, g_hat) <- relu(o', g); clamp01(relu(o'))==clamp01(o')
                nc.scalar.activation(state[:, 0:2, si, :],
                                     pgs[si][:, 2:4, tl, :], AF.Relu)
            for si in range(NS):
                # (t1, cf) = clamp01((i', f')) * (g_hat, c)
                nc.vector._custom_dve(CLAMP_MUL, out=T[:, :, si, :],
                                      in0=pgs[si][:, 0:2, tl, :],
                                      in1=state[:, 1:3, si, :])
            for si in range(NS):
                nc.gpsimd.tensor_tensor(state[:, 2, si, :], T[:, 0, si, :],
                                        T[:, 1, si, :], OP.add)
            for si in range(NS):
                h_aps[si] = Hout[:, t_off + t, si * SB:(si + 1) * SB]
                nc.vector._custom_dve(HSIG_MUL, out=h_aps[si],
                                      in0=state[:, 2, si, :],
                                      in1=state[:, 0, si, :],
                                      s0=0.25, s1=0.5)
    return h_aps


def _conv_b(nc, pools, dst, src, lhsT, taps, n_len, scale, bias, out_parts,
            y_b=None, ptag="pg0"):
    """Per-sample K-tap conv: dst [parts, PAD+n_len+PAD] (halo-padded 2D tile)
    = relu(scale*psum+bias). If y_b is given (d22), DMA each tile to DRAM."""
    psum_pool = pools["psum_d"]
    for n0 in range(0, n_len, 512):
        pd = psum_pool.tile([out_parts, 512], F32, tag=ptag, name="pd")
        for i, (delta, blk) in enumerate(taps):
            rhs = src[:, PAD + n0 + delta: PAD + n0 + delta + 512]
            nc.tensor.matmul(pd[:], lhsT[:, blk * out_parts:(blk + 1) * out_parts],
                             rhs, start=(i == 0), stop=(i == len(taps) - 1))
        if y_b is not None:
            ot = pools["otile"].tile([out_parts, 512], F32, tag="otile",
                                     name="ot")
            nc.scalar.activation(ot[:], pd[:], AF.Relu, bias=bias[:],
                                 scale=scale[:])
            nc.sync.dma_start(y_b[:, n0:n0 + 512], ot[:])
        else:
            nc.scalar.activation(dst[:, PAD + n0:PAD + n0 + 512], pd[:],
                                 AF.Relu, bias=bias[:], scale=scale[:])


def _convT_b(nc, pools, dst, srcs, lhsT, out_parts, m_len, ptag="pg0"):
    """Per-sample convT stride 2: dst[:, PAD+2m+r] accumulated over taps and
    input chunks. srcs: list of [parts, PAD+m_len+PAD] APs."""
    psum_pool = pools["psum_d"]
    nchunk = len(srcs)
    dv = dst.rearrange("p (m r) -> p m r", r=2)
    for r in (0, 1):
        for m0 in range(0, m_len, 512):
            pd = psum_pool.tile([out_parts, 512], F32, tag=ptag, name="pd")
            first = True
            for ti, (delta, _) in enumerate(CONVT_TAPS[r]):
                for ch in range(nchunk):
                    blk = (r * 4 + ti) * nchunk + ch
                    rhs = srcs[ch][:, PAD + m0 + delta: PAD + m0 + delta + 512]
                    nc.tensor.matmul(
                        pd[:], lhsT[:, blk * out_parts:(blk + 1) * out_parts],
                        rhs, start=first, stop=(ti == 3 and ch == nchunk - 1))
                    first = False
            # dst col = PAD + 2(m0+m) + r = 2*(PAD//2 + m0 + m) + r
            nc.vector.tensor_copy(dv[:, PAD // 2 + m0: PAD // 2 + m0 + 512, r],
                                  pd[:])


def build_nc():
    nc = bacc.Bacc()
    xcol_d = nc.declare_dram_parameter("xcol", [8, BL * L], BF16, isOutput=False)
    wd = {name: nc.declare_dram_parameter(name, shape, dt, isOutput=False)
          for name, shape, dt in _WSPEC}
    y_d = nc.declare_dram_parameter("y", [BL, 4, L2], F32, isOutput=True)

    with tile.TileContext(nc) as tc:
        with ExitStack() as top:
            wpool = top.enter_context(tc.tile_pool(name="w", bufs=1))
            state = top.enter_context(tc.tile_pool(name="state", bufs=1))
            tmp = top.enter_context(tc.tile_pool(name="tmp", bufs=4))
            psum_r = top.enter_context(tc.tile_pool(name="psum_r", bufs=2,
                                                    space="PSUM"))
            psum_d = psum_r
            otile = top.enter_context(tc.tile_pool(name="otile", bufs=3))

            wt = {}
            for name, shape, dt in _WSPEC:
                wt[name] = wpool.tile(shape, dt, tag=f"w_{name}", name=f"w_{name}")
                nc.sync.dma_start(wt[name][:], wd[name][:])
            xcol = wpool.tile([8, BL * L], BF16, tag="xcol")
            nc.sync.dma_start(xcol[:], xcol_d[:])

            ind = wt["ind"]

            st = state.tile([HS, 3, NS, SB], F32, tag="lstm_state",
                            name="lstm_state")
            nc.gpsimd.memset(st[:, 2, :, :], 0.0)
            hz = state.tile([HS, BL], BF16, tag="hz")
            nc.gpsimd.memset(hz[:], 0.0)

            pools = {"psum_r": psum_r, "psum_d": psum_d, "tmp": tmp,
                     "ind": ind, "otile": otile, "y": y_d}

            bigpool = top.enter_context(tc.tile_pool(name="big", bufs=1))
            dpool = top.enter_context(tc.tile_pool(name="dec", bufs=3))

            E = bigpool.tile([HS, L, BL], BF16, tag="E")
            for b in range(BL):
                pe = psum_d.tile([HS, 512], F32, tag="pg0")
                nc.tensor.matmul(pe[:], wt["w1"][:], xcol[:, b * L:(b + 1) * L],
                                 start=True, stop=True)
                nc.scalar.activation(E[:, :, b], pe[:], AF.Relu,
                                     bias=wt["b1"][:], scale=wt["s1"][:])

            H0F = bigpool.tile([HS, L, BL], BF16, tag="H0F")
            H0R = bigpool.tile([HS, L, BL], BF16, tag="H0R")
            h_aps = [hz[:, si * SB:(si + 1) * SB] for si in range(NS)]
            h_aps = _emit_pass(nc, pools,
                               {"Wa": wt["W0fa"], "U": wt["U0f"],
                                "bA": wt["bA0f"], "bB": wt["bB0f"]},
                               [E[:]], H0F, h_aps, st, False, 0)
            h_aps = _emit_pass(nc, pools,
                               {"Wa": wt["W0ra"], "U": wt["U0r"],
                                "bA": wt["bA0r"], "bB": wt["bB0r"]},
                               [E[:]], H0R, h_aps, st, True, 0)

            LS = L + 2 * PAD
            H1F = bigpool.tile([HS, LS, BL], BF16, tag="H1F")
            H1R = bigpool.tile([HS, LS, BL], BF16, tag="H1R")
            for Hb in (H1F, H1R):
                nc.gpsimd.memset(Hb[:, 0:PAD, :], 0.0)
                nc.gpsimd.memset(Hb[:, PAD + L:LS, :], 0.0)
            h_aps = _emit_pass(nc, pools,
                               {"Wa": wt["W1fa"], "Wb": wt["W1fb"],
                                "U": wt["U1f"], "bA": wt["bA1f"],
                                "bB": wt["bB1f"]},
                               [H0F[:], H0R[:]], H1F, h_aps, st, False, PAD)
            h_aps = _emit_pass(nc, pools,
                               {"Wa": wt["W1ra"], "Wb": wt["W1rb"],
                                "U": wt["U1r"], "bA": wt["bA1r"],
                                "bB": wt["bB1r"]},
                               [H0F[:], H0R[:]], H1R, h_aps, st, True, PAD)

            # ---- decoder: per-sample chain with small ping-pong tiles ----
            S1 = L1 + 2 * PAD
            S2 = L2 + 2 * PAD
            for b in range(BL):
                D1 = dpool.tile([HS, S1], BF16, tag="D1", name="D1")
                nc.gpsimd.memset(D1[:, 0:PAD], 0.0)
                nc.gpsimd.memset(D1[:, PAD + L1:S1], 0.0)
                _convT_b(nc, pools, D1,
                         [H1F[:, :, b], H1R[:, :, b]], wt["u1"], HS, L,
                         ptag="pg0")
                D2 = dpool.tile([HS, S1], BF16, tag="D2", name="D2")
                nc.gpsimd.memset(D2[:, 0:PAD], 0.0)
                nc.gpsimd.memset(D2[:, PAD + L1:S1], 0.0)
                _conv_b(nc, pools, D2, D1, wt["d11"], K5_TAPS, L1,
                        wt["s11"], wt["b11"], HS, ptag="pg1")
                D3 = dpool.tile([64, S1], BF16, tag="D3", name="D3")
                nc.gpsimd.memset(D3[:, 0:PAD], 0.0)
                nc.gpsimd.memset(D3[:, PAD + L1:S1], 0.0)
                _conv_b(nc, pools, D3, D2, wt["d12"], K5_TAPS, L1,
                        wt["s12"], wt["b12"], 64, ptag="pg0")
                D4 = dpool.tile([32, S2], BF16, tag="D4", name="D4")
                nc.gpsimd.memset(D4[:, 0:PAD], 0.0)
                nc.gpsimd.memset(D4[:, PAD + L2:S2], 0.0)
                _convT_b(nc, pools, D4, [D3], wt["u2"], 32, L1, ptag="pg1")
                D5 = dpool.tile([32, S2], BF16, tag="D5", name="D5")
                nc.gpsimd.memset(D5[:, 0:PAD], 0.0)
                nc.gpsimd.memset(D5[:, PAD + L2:S2], 0.0)
                _conv_b(nc, pools, D5, D4, wt["d21"], K5_TAPS, L2,
                        wt["s21"], wt["b21"], 32, ptag="pg0")
                _conv_b(nc, pools, None, D5, wt["d22"], K5_TAPS, L2,
                        wt["s22"], wt["b22"], 4, y_b=y_d[b], ptag="pg1")
    nc.finalize()
    return nc


_NC = None


def _get_nc():
    global _NC
    if _NC is None:
        _NC = build_nc()
    return _NC


def kernel(**inputs):
    nc = _get_nc()
    w = _prep_weights(inputs)
    x = np.asarray(inputs["x"], np.float32)
    in_maps = []
    for c in range(N_CORES):
        m = dict(w)
        m["xcol"] = _prep_xcol(x[c * BL:(c + 1) * BL])
        in_maps.append(m)
    trace = bool(int(os.environ.get("BASS_KERNEL_TRACE", "0")))
    res = run_bass_kernel_spmd(nc, in_maps, list(range(N_CORES)), trace=trace)
    if trace:
        kernel.last_exec_time_ns = res.exec_time_ns
    out = np.concatenate([res.results[i]["y"] for i in range(N_CORES)], axis=0)
    return np.ascontiguousarray(out.astype(np.float32))



# revision 15
# speedup vs baseline: 1.2903x; 1.0601x over previous
"""Trainium2 Bass kernel for CNN-BiLSTM encoder/decoder (nn_CNN_BiLSTM_AttenQ).

Data-parallel over batch: B=128 sharded 8 ways (16 samples/core), weights
replicated, no collectives. Per core:
  encoder conv (matmul over host-im2col patches) ->
  4 sequential LSTM passes (2 layers x 2 dirs, h/c carried across passes) ->
  decoder conv stack (convs/convTs as tap-accumulated matmuls).

Layouts: activations channel-on-partition, batch-major free [C, (b, t)].
LSTM gates computed transposed: PSUM [128, 4, CH, BL] with gate chunk order
[i, f, o, g]; i,f,o pre-scaled by 0.25 (+0.5 via bias preload) so
hard-sigmoid == clamp01.
"""

import os
import numpy as np
import ml_dtypes

import concourse.mybir as mybir
import concourse.tile as tile
from concourse import bacc
from concourse import dve_ops
from concourse.dve_spec import (Spec, Src0, Src1, C0, C1, One, relu, minn,
                                lower, _has_src1)
from concourse.dve_uop import DveOpSpec
from concourse.bass_utils import run_bass_kernel_spmd
from contextlib import ExitStack


def _register_dve_op(name, body, ref):
    """Author a custom DVE op at runtime (sha pinned from our own lowering)."""
    for op in dve_ops.OPS:
        if op.name == name:
            return op
    spec = Spec(body=body, reference=ref)
    op = dve_ops.DveOp(name, spec, subdim=False, uops_sha={})
    dve_ops._SUB_OPCODE_FOR_NAME[name] = max(dve_ops._SUB_OPCODE_FOR_NAME.values()) + 1
    dve_ops.OPS.append(op)
    dve_ops.CUSTOM_DVE_SPECS[name] = spec
    for ver in ("v3", "v4"):
        uops = lower(spec, ver=ver)
        op.uops_sha[ver] = DveOpSpec(
            name=name, opcode=dve_ops.get_dve_sub_opcode(name), uops=uops,
            rd1_en=_has_src1(spec)).sha(ver)
    return op


# out = clamp01(in0) * in1
CLAMP_MUL = _register_dve_op(
    "ANT_CLAMP_MUL", minn(relu(Src0), One) * Src1,
    lambda in0, in1, s0, s1, imm2: np.minimum(np.maximum(in0, 0), 1) * in1)
# out = clamp01(in0*s0 + s1) * clamp01(in1)
HSIG_MUL = _register_dve_op(
    "ANT_HSIG_MUL",
    minn(relu(Src0 * C0 + C1), One) * minn(relu(Src1), One),
    lambda in0, in1, s0, s1, imm2: np.minimum(np.maximum(in0 * s0 + s1, 0), 1)
    * np.minimum(np.maximum(in1, 0), 1))

F32 = mybir.dt.float32
BF16 = mybir.dt.bfloat16
AF = mybir.ActivationFunctionType
OP = mybir.AluOpType
BFNP = ml_dtypes.bfloat16

B, T, C, HS = 128, 2048, 128, 128
N_CORES = 8
BL = B // N_CORES          # 16 samples per core
L = T // 4                 # 512 encoder output length
CH = 32                    # recurrence chunk length (timesteps)
NCH = L // CH
L1 = 2 * L                 # 1024
L2 = T                     # 2048
PAD = 4                    # halo pad per batch segment in decoder buffers
EPS = 1e-5

# convT taps: out[2m+r] += x[m+delta] @ w[:, :, k]  -> list of (delta, k)
CONVT_TAPS = {0: [(1, 1), (0, 3), (-1, 5), (-2, 7)],
              1: [(2, 0), (1, 2), (0, 4), (-1, 6)]}
K5_TAPS = [(k - 2, k) for k in range(5)]


def _bf(x):
    return np.ascontiguousarray(np.asarray(x, np.float32).astype(BFNP))


def _f32(x):
    return np.ascontiguousarray(np.asarray(x, np.float32))


def _prep_dir(W, U, b):
    perm = [0, 1, 3, 2]  # i,f,g,o -> i,f,o,g
    Wp = np.concatenate([W[:, j * HS:(j + 1) * HS] for j in perm], 1).astype(np.float64)
    Up = np.concatenate([U[:, j * HS:(j + 1) * HS] for j in perm], 1).astype(np.float64)
    bp = np.concatenate([b[j * HS:(j + 1) * HS] for j in perm]).astype(np.float64)
    Wp[:, :3 * HS] *= 0.25
    Up[:, :3 * HS] *= 0.25
    bp[:3 * HS] = 0.25 * bp[:3 * HS] + 0.5
    bA = np.stack([bp[0:HS], bp[HS:2 * HS]])
    bB = np.stack([bp[2 * HS:3 * HS], bp[3 * HS:4 * HS]])
    return _bf(Wp), _bf(Up), _bf(bA), _bf(bB)


def _prep_weights(inp):
    w = {}
    w["w1"] = _bf(np.asarray(inp["conv1_w"])[:, 0, :].T)            # [8,128]
    w["s1"] = _f32(np.asarray(inp["bn1_g"]) / np.sqrt(1.0 + EPS))[:, None]
    w["b1"] = _f32(np.asarray(inp["bn1_b"]))[:, None]
    for tag, (Wk, Uk, bk) in {
        "0f": ("W0f", "U0f", "b0f"), "0r": ("W0r", "U0r", "b0r"),
        "1f": ("W1f", "U1f", "b1f"), "1r": ("W1r", "U1r", "b1r"),
    }.items():
        Wp, Up, bA, bB = _prep_dir(np.asarray(inp[Wk], np.float32),
                                   np.asarray(inp[Uk], np.float32),
                                   np.asarray(inp[bk], np.float32))
        if Wp.shape[0] == 2 * HS:
            w[f"W{tag}a"] = np.ascontiguousarray(Wp[:HS])
            w[f"W{tag}b"] = np.ascontiguousarray(Wp[HS:])
        else:
            w[f"W{tag}a"] = Wp
        w[f"U{tag}"], w[f"bA{tag}"], w[f"bB{tag}"] = Up, bA, bB
    u1w = np.asarray(inp["up1_w"], np.float32)                      # [256,128,8]
    u1 = np.zeros((HS, 16 * HS), np.float32)
    for r in (0, 1):
        for ti, (_, k) in enumerate(CONVT_TAPS[r]):
            for ch in (0, 1):
                blk = (r * 4 + ti) * 2 + ch
                u1[:, blk * HS:(blk + 1) * HS] = u1w[ch * HS:(ch + 1) * HS, :, k]
    w["u1"] = _bf(u1)
    d11w = np.asarray(inp["d11_w"], np.float32)
    w["d11"] = _bf(np.concatenate([d11w[:, :, k].T for k in range(5)], 1))
    w["s11"] = _f32(np.asarray(inp["bn11_g"]) / np.sqrt(1.0 + EPS))[:, None]
    w["b11"] = _f32(np.asarray(inp["bn11_b"]))[:, None]
    d12w = np.asarray(inp["d12_w"], np.float32)
    w["d12"] = _bf(np.concatenate([d12w[:, :, k].T for k in range(5)], 1))
    w["s12"] = _f32(np.asarray(inp["bn12_g"]) / np.sqrt(1.0 + EPS))[:, None]
    w["b12"] = _f32(np.asarray(inp["bn12_b"]))[:, None]
    u2w = np.asarray(inp["up2_w"], np.float32)                      # [64,32,8]
    u2 = np.zeros((64, 8 * 32), np.float32)
    for r in (0, 1):
        for ti, (_, k) in enumerate(CONVT_TAPS[r]):
            blk = r * 4 + ti
            u2[:, blk * 32:(blk + 1) * 32] = u2w[:, :, k]
    w["u2"] = _bf(u2)
    d21w = np.asarray(inp["d21_w"], np.float32)
    w["d21"] = _bf(np.concatenate([d21w[:, :, k].T for k in range(5)], 1))
    w["s21"] = _f32(np.asarray(inp["bn21_g"]) / np.sqrt(1.0 + EPS))[:, None]
    w["b21"] = _f32(np.asarray(inp["bn21_b"]))[:, None]
    d22w = np.asarray(inp["d22_w"], np.float32)
    w["d22"] = _bf(np.concatenate([d22w[:, :, k].T for k in range(5)], 1))
    w["s22"] = _f32(np.asarray(inp["bn22_g"]) / np.sqrt(1.0 + EPS))[:, None]
    w["b22"] = _f32(np.asarray(inp["bn22_b"]))[:, None]
    blk = CH * (BL // NS)          # 256 cols per gate block
    ind = np.zeros((2, 2 * blk), np.float32)
    ind[0, :blk] = 1.0
    ind[1, blk:] = 1.0
    w["ind"] = _bf(ind)
    return w


def _prep_xcol(x_shard):
    xp = np.pad(np.asarray(x_shard, np.float32)[:, 0, :], ((0, 0), (3, 4)))
    cols = np.stack([xp[:, k:k + 4 * L:4] for k in range(8)])       # [8,BL,L]
    return _bf(cols.reshape(8, BL * L))


_WSPEC = [
    ("w1", [8, HS], BF16), ("s1", [HS, 1], F32), ("b1", [HS, 1], F32),
    ("W0fa", [HS, 512], BF16), ("U0f", [HS, 512], BF16),
    ("bA0f", [2, HS], BF16), ("bB0f", [2, HS], BF16),
    ("W0ra", [HS, 512], BF16), ("U0r", [HS, 512], BF16),
    ("bA0r", [2, HS], BF16), ("bB0r", [2, HS], BF16),
    ("W1fa", [HS, 512], BF16), ("W1fb", [HS, 512], BF16), ("U1f", [HS, 512], BF16),
    ("bA1f", [2, HS], BF16), ("bB1f", [2, HS], BF16),
    ("W1ra", [HS, 512], BF16), ("W1rb", [HS, 512], BF16), ("U1r", [HS, 512], BF16),
    ("bA1r", [2, HS], BF16), ("bB1r", [2, HS], BF16),
    ("u1", [HS, 16 * HS], BF16),
    ("d11", [HS, 640], BF16), ("s11", [HS, 1], F32), ("b11", [HS, 1], F32),
    ("d12", [HS, 320], BF16), ("s12", [64, 1], F32), ("b12", [64, 1], F32),
    ("u2", [64, 256], BF16),
    ("d21", [32, 160], BF16), ("s21", [32, 1], F32), ("b21", [32, 1], F32),
    ("d22", [32, 20], BF16), ("s22", [4, 1], F32), ("b22", [4, 1], F32),
    ("ind", [2, 512], BF16),
]


NS = 2                      # phase-offset streams per core
SB = BL // NS               # 8 samples per stream


def _emit_pass(nc, pools, wt, ins, Hout, h_aps, state, reverse, t_off):
    """One LSTM direction (L steps, NCH chunks), NS interleaved batch
    streams. ins: input APs [128,L,BL] (t-major). Hout: [128,Lseg,BL]; h
    written at [:,t_off+t,bs]. state: persistent [HS,3,NS,SB] fp32 SBUF
    tile holding (o_hat, g_hat, c) rows per stream. Returns updated h_aps.

    Per step per stream the work is spread over engines:
      PE:     4 U-gate MMs (streams paired per gate to share LDWEIGHTS)
      scalar: (o_hat, g_hat) = relu(pg[o,g])  PSUM->SBUF, one op
      DVE:    (t1, cf) = clamp01(pg[i,f]) * (g_hat, c)  one paired op
      gpsimd: c = t1 + cf  (SBUF only)
      DVE:    h = hsig(c) * clamp01(o_hat)  all-SBUF op
    DVE queue order is [V1_s0, V1_s1, V2_s0, V2_s1] so stream 1's V1 is
    not blocked behind stream 0's pool-dependent V2 (lets streams slide).
    """
    psum_pool, tmp = pools["psum_r"], pools["tmp"]
    ind = pools["ind"]
    n_in = 2 if "Wb" in wt else 1
    h_aps = list(h_aps)

    def gemm_pieces(ci, si):
        """Allocate pg and return (pg, T0, [thunk, ...]) — small PE emission
        pieces to spread between steps (PE is in-order; keep pieces short)."""
        T0 = ci * CH if not reverse else L - (ci + 1) * CH
        pg = psum_pool.tile([HS, 4, CH, SB], F32, tag=f"pg{si}",
                            name=f"pg{si}")

        def bias():
            nc.tensor.matmul(pg[:, 0:2, :, :], wt["bA"][:], ind[:],
                             start=True, stop=False)
            nc.tensor.matmul(pg[:, 2:4, :, :], wt["bB"][:], ind[:],
                             start=True, stop=False)

        pieces = [bias]
        for j in range(4):
            for idx in range(n_in):
                def wmm(j=j, idx=idx):
                    Wm = wt["Wa"] if idx == 0 else wt["Wb"]
                    rhs = ins[idx][:, T0:T0 + CH, si * SB:(si + 1) * SB]
                    nc.tensor.matmul(pg[:, j, :, :],
                                     Wm[:, j * HS:(j + 1) * HS],
                                     rhs, start=False, stop=False)
                pieces.append(wmm)
        return pg, T0, pieces

    def run_all(pieces_list):
        for _, _, pieces in pieces_list:
            for p in pieces:
                p()

    # gate order in pg: [i, f, o, g]; emit g first so the scalar relu (which
    # is on the critical chain) starts as early as possible; o last (V2 reads
    # it straight from PSUM late in the chain).
    GATE_SEQ = (3, 0, 1, 2)

    nxt = [gemm_pieces(0, si) for si in range(NS)]
    run_all(nxt)
    for ci in range(NCH):
        pgs = [pg for pg, _, _ in nxt]
        T0 = nxt[0][1]
        if ci + 1 < NCH:
            nxt = [gemm_pieces(ci + 1, si) for si in range(NS)]
            todo = [p for _, _, pieces in nxt for p in pieces]
        else:
            nxt, todo = [], []
        for s in range(CH):
            tl = s if not reverse else CH - 1 - s
            t = T0 + tl
            T = tmp.tile([HS, 2, NS, SB], F32, tag="T", name="T")
            for j in GATE_SEQ:
                for si in range(NS):
                    nc.tensor.matmul(pgs[si][:, j, tl, :],
                                     wt["U"][:, j * HS:(j + 1) * HS],
                                     h_aps[si], start=False,
                                     stop=(j == GATE_SEQ[-1]))
            # one W@x piece of the next chunk per step, early in the slot
            if s < len(todo):
                todo[s]()
            for si in range(NS):
                # g_hat <- relu(g); on the critical chain, so g's MM is first
                nc.scalar.activation(state[:, 1, si, :],
                                     pgs[si][:, 3, tl, :], AF.Relu)
            for si in range(NS):
                # (t1, cf) = clamp01((i', f')) * (g_hat, c); then c = t1+cf
                # and h = hsig(c)*clamp01(o') back-to-back on the DVE queue
                # (no cross-engine handoffs on the chain).
                nc.vector._custom_dve(CLAMP_MUL, out=T[:, :, si, :],
                                      in0=pgs[si][:, 0:2, tl, :],
                                      in1=state[:, 1:3, si, :])
                nc.vector.tensor_tensor(state[:, 2, si, :], T[:, 0, si, :],
                                        T[:, 1, si, :], OP.add)
                h_aps[si] = Hout[:, t_off + t, si * SB:(si + 1) * SB]
                nc.vector._custom_dve(HSIG_MUL, out=h_aps[si],
                                      in0=state[:, 2, si, :],
                                      in1=pgs[si][:, 2, tl, :],
                                      s0=0.25, s1=0.5)
    return h_aps


def _conv_b(nc, pools, dst, src, lhsT, taps, n_len, scale, bias, out_parts,
            y_b=None, ptag="pg0"):
    """Per-sample K-tap conv: dst [parts, PAD+n_len+PAD] (halo-padded 2D tile)
    = relu(scale*psum+bias). If y_b is given (d22), DMA each tile to DRAM."""
    psum_pool = pools["psum_d"]
    for n0 in range(0, n_len, 512):
        pd = psum_pool.tile([out_parts, 512], F32, tag=ptag, name="pd")
        for i, (delta, blk) in enumerate(taps):
            rhs = src[:, PAD + n0 + delta: PAD + n0 + delta + 512]
            nc.tensor.matmul(pd[:], lhsT[:, blk * out_parts:(blk + 1) * out_parts],
                             rhs, start=(i == 0), stop=(i == len(taps) - 1))
        if y_b is not None:
            ot = pools["otile"].tile([out_parts, 512], F32, tag="otile",
                                     name="ot")
            nc.scalar.activation(ot[:], pd[:], AF.Relu, bias=bias[:],
                                 scale=scale[:])
            nc.sync.dma_start(y_b[:, n0:n0 + 512], ot[:])
        else:
            nc.scalar.activation(dst[:, PAD + n0:PAD + n0 + 512], pd[:],
                                 AF.Relu, bias=bias[:], scale=scale[:])


def _convT_b(nc, pools, dst, srcs, lhsT, out_parts, m_len, ptag="pg0"):
    """Per-sample convT stride 2: dst[:, PAD+2m+r] accumulated over taps and
    input chunks. srcs: list of [parts, PAD+m_len+PAD] APs."""
    psum_pool = pools["psum_d"]
    nchunk = len(srcs)
    dv = dst.rearrange("p (m r) -> p m r", r=2)
    for r in (0, 1):
        for m0 in range(0, m_len, 512):
            pd = psum_pool.tile([out_parts, 512], F32, tag=ptag, name="pd")
            first = True
            for ti, (delta, _) in enumerate(CONVT_TAPS[r]):
                for ch in range(nchunk):
                    blk = (r * 4 + ti) * nchunk + ch
                    rhs = srcs[ch][:, PAD + m0 + delta: PAD + m0 + delta + 512]
                    nc.tensor.matmul(
                        pd[:], lhsT[:, blk * out_parts:(blk + 1) * out_parts],
                        rhs, start=first, stop=(ti == 3 and ch == nchunk - 1))
                    first = False
            # dst col = PAD + 2(m0+m) + r = 2*(PAD//2 + m0 + m) + r
            nc.vector.tensor_copy(dv[:, PAD // 2 + m0: PAD // 2 + m0 + 512, r],
                                  pd[:])


def build_nc():
    nc = bacc.Bacc()
    xcol_d = nc.declare_dram_parameter("xcol", [8, BL * L], BF16, isOutput=False)
    wd = {name: nc.declare_dram_parameter(name, shape, dt, isOutput=False)
          for name, shape, dt in _WSPEC}
    y_d = nc.declare_dram_parameter("y", [BL, 4, L2], F32, isOutput=True)

    with tile.TileContext(nc) as tc:
        with ExitStack() as top:
            wpool = top.enter_context(tc.tile_pool(name="w", bufs=1))
            state = top.enter_context(tc.tile_pool(name="state", bufs=1))
            tmp = top.enter_context(tc.tile_pool(name="tmp", bufs=4))
            psum_r = top.enter_context(tc.tile_pool(name="psum_r", bufs=2,
                                                    space="PSUM"))
            psum_d = psum_r
            otile = top.enter_context(tc.tile_pool(name="otile", bufs=3))

            wt = {}
            for name, shape, dt in _WSPEC:
                wt[name] = wpool.tile(shape, dt, tag=f"w_{name}", name=f"w_{name}")
                nc.sync.dma_start(wt[name][:], wd[name][:])
            xcol = wpool.tile([8, BL * L], BF16, tag="xcol")
            nc.sync.dma_start(xcol[:], xcol_d[:])

            ind = wt["ind"]

            st = state.tile([HS, 3, NS, SB], F32, tag="lstm_state",
                            name="lstm_state")
            nc.gpsimd.memset(st[:, 2, :, :], 0.0)
            hz = state.tile([HS, BL], BF16, tag="hz")
            nc.gpsimd.memset(hz[:], 0.0)

            pools = {"psum_r": psum_r, "psum_d": psum_d, "tmp": tmp,
                     "ind": ind, "otile": otile, "y": y_d}

            # ~9us of dense dummy matmuls so the PE HAM un-throttles to
            # K=8/8 before the latency-critical recurrence begins; the
            # steady MM trickle afterwards never idles long enough to
            # re-throttle.
            wu = psum_r.tile([HS, 512], F32, tag="pg0", name="warm")
            for _ in range(40):
                nc.tensor.matmul(wu[:], wt["U0f"][:, 0:HS],
                                 wt["u1"][:, 0:512], start=True, stop=True)

            bigpool = top.enter_context(tc.tile_pool(name="big", bufs=1))
            dpool = top.enter_context(tc.tile_pool(name="dec", bufs=3))

            E = bigpool.tile([HS, L, BL], BF16, tag="E")
            for b in range(BL):
                pe = psum_d.tile([HS, 512], F32, tag="pg0")
                nc.tensor.matmul(pe[:], wt["w1"][:], xcol[:, b * L:(b + 1) * L],
                                 start=True, stop=True)
                nc.scalar.activation(E[:, :, b], pe[:], AF.Relu,
                                     bias=wt["b1"][:], scale=wt["s1"][:])

            H0F = bigpool.tile([HS, L, BL], BF16, tag="H0F")
            H0R = bigpool.tile([HS, L, BL], BF16, tag="H0R")
            h_aps = [hz[:, si * SB:(si + 1) * SB] for si in range(NS)]
            h_aps = _emit_pass(nc, pools,
                               {"Wa": wt["W0fa"], "U": wt["U0f"],
                                "bA": wt["bA0f"], "bB": wt["bB0f"]},
                               [E[:]], H0F, h_aps, st, False, 0)
            h_aps = _emit_pass(nc, pools,
                               {"Wa": wt["W0ra"], "U": wt["U0r"],
                                "bA": wt["bA0r"], "bB": wt["bB0r"]},
                               [E[:]], H0R, h_aps, st, True, 0)

            LS = L + 2 * PAD
            H1F = bigpool.tile([HS, LS, BL], BF16, tag="H1F")
            H1R = bigpool.tile([HS, LS, BL], BF16, tag="H1R")
            for Hb in (H1F, H1R):
                nc.gpsimd.memset(Hb[:, 0:PAD, :], 0.0)
                nc.gpsimd.memset(Hb[:, PAD + L:LS, :], 0.0)
            h_aps = _emit_pass(nc, pools,
                               {"Wa": wt["W1fa"], "Wb": wt["W1fb"],
                                "U": wt["U1f"], "bA": wt["bA1f"],
                                "bB": wt["bB1f"]},
                               [H0F[:], H0R[:]], H1F, h_aps, st, False, PAD)
            h_aps = _emit_pass(nc, pools,
                               {"Wa": wt["W1ra"], "Wb": wt["W1rb"],
                                "U": wt["U1r"], "bA": wt["bA1r"],
                                "bB": wt["bB1r"]},
                               [H0F[:], H0R[:]], H1R, h_aps, st, True, PAD)

            # ---- decoder: per-sample chain with small ping-pong tiles ----
            S1 = L1 + 2 * PAD
            S2 = L2 + 2 * PAD
            for b in range(BL):
                D1 = dpool.tile([HS, S1], BF16, tag="D1", name="D1")
                nc.gpsimd.memset(D1[:, 0:PAD], 0.0)
                nc.gpsimd.memset(D1[:, PAD + L1:S1], 0.0)
                _convT_b(nc, pools, D1,
                         [H1F[:, :, b], H1R[:, :, b]], wt["u1"], HS, L,
                         ptag="pg0")
                D2 = dpool.tile([HS, S1], BF16, tag="D2", name="D2")
                nc.gpsimd.memset(D2[:, 0:PAD], 0.0)
                nc.gpsimd.memset(D2[:, PAD + L1:S1], 0.0)
                _conv_b(nc, pools, D2, D1, wt["d11"], K5_TAPS, L1,
                        wt["s11"], wt["b11"], HS, ptag="pg1")
                D3 = dpool.tile([64, S1], BF16, tag="D3", name="D3")
                nc.gpsimd.memset(D3[:, 0:PAD], 0.0)
                nc.gpsimd.memset(D3[:, PAD + L1:S1], 0.0)
                _conv_b(nc, pools, D3, D2, wt["d12"], K5_TAPS, L1,
                        wt["s12"], wt["b12"], 64, ptag="pg0")
                D4 = dpool.tile([32, S2], BF16, tag="D4", name="D4")
                nc.gpsimd.memset(D4[:, 0:PAD], 0.0)
                nc.gpsimd.memset(D4[:, PAD + L2:S2], 0.0)
                _convT_b(nc, pools, D4, [D3], wt["u2"], 32, L1, ptag="pg1")
                D5 = dpool.tile([32, S2], BF16, tag="D5", name="D5")
                nc.gpsimd.memset(D5[:, 0:PAD], 0.0)
                nc.gpsimd.memset(D5[:, PAD + L2:S2], 0.0)
                _conv_b(nc, pools, D5, D4, wt["d21"], K5_TAPS, L2,
                        wt["s21"], wt["b21"], 32, ptag="pg0")
                _conv_b(nc, pools, None, D5, wt["d22"], K5_TAPS, L2,
                        wt["s22"], wt["b22"], 4, y_b=y_d[b], ptag="pg1")
    nc.finalize()
    return nc


_NC = None


def _get_nc():
    global _NC
    if _NC is None:
        _NC = build_nc()
    return _NC


def kernel(**inputs):
    nc = _get_nc()
    w = _prep_weights(inputs)
    x = np.asarray(inputs["x"], np.float32)
    in_maps = []
    for c in range(N_CORES):
        m = dict(w)
        m["xcol"] = _prep_xcol(x[c * BL:(c + 1) * BL])
        in_maps.append(m)
    trace = bool(int(os.environ.get("BASS_KERNEL_TRACE", "0")))
    res = run_bass_kernel_spmd(nc, in_maps, list(range(N_CORES)), trace=trace)
    if trace:
        kernel.last_exec_time_ns = res.exec_time_ns
    out = np.concatenate([res.results[i]["y"] for i in range(N_CORES)], axis=0)
    return np.ascontiguousarray(out.astype(np.float32))

